# revision 2
# baseline (speedup 1.0000x reference)
"""TRN2 Bass kernel v2 for nn_CrossAttention (N=4, Lq=Lkv=2048, H=16, hd=64).

Sharding: 8 cores = (batch b = core//2) x (query-length half = core%2).
Each core: cross-attention for 1024 query rows of one batch, all 2048 kv.

Host-side prep: inputs are pre-transposed and cast to bf16 (xqT [DQ, LQ],
xkvT [DKV, LKV]) so no on-chip transpose phase is needed.

Pipeline (per core):
  B(p): projections for head-pair p (Q^T, K^T pair cols; V natural+ones)
  C(p): attention for pair p: per (qb, kc-pair): 4 S-matmuls (2-way row
        tiled, K=64 strips at partitions 0/64), exp split ACT(exact)/
        DVE(Schraudolph u16->bf16), 4 PV matmuls accumulating po[128,512]
        (rows 0:64 = O^T, rows 64:128 = softmax denom via ones cols).
        Normalize with DVE bitcast-reciprocal + Newton.
  B(p+1) matmuls are interleaved into C(p) so PE never starves while
  ACT/DVE chew on exp.
  D: out-projection OTn^T @ Wo + bias.
"""

import math
import os

import numpy as np

import concourse.bass as bass
import concourse.mybir as mybir
import concourse.tile as tile
from concourse import bacc
from concourse.bass_utils import run_bass_kernel_spmd

F32 = mybir.dt.float32
BF16 = mybir.dt.bfloat16
F8 = mybir.dt.float8e4
U16 = mybir.dt.uint16
I32 = mybir.dt.int32
AF = mybir.ActivationFunctionType
OP = mybir.AluOpType

DQ = 1024      # query feature dim
DKV = 768      # kv feature dim
LQ = 1024      # per-core query rows
LKV = 2048     # kv rows
H = 16         # heads
HD = 64        # head dim
OD = 1024      # output dim
NPAIR = 8      # head pairs (128 cols each)
SCALE = HD ** -0.5

NDCQ = DQ // 128    # 8 d-chunks for query features
NDCK = DKV // 128   # 6 d-chunks for kv features
NKC = LKV // 128    # 16 lkv chunks
NB = NKC // 2       # 8 kc-pairs per (pair, qb) iteration
NQB = LQ // 512     # 2 q blocks

# Schraudolph exp constants: u16 = round(A16*(SCALE*s) + C16), bitcast bf16
A16 = 128.0 / math.log(2.0)
C16 = 16250.65  # optimal ~16250.4 (round-nearest) / ~16250.9 (truncate)
# ACT share of exp ops out of 16 (rest go to DVE)
ACT_SHARE16 = int(os.environ.get("BASS_ACT_SHARE", "8"))


def build(nc: bass.Bass):
    xqT = nc.dram_tensor("xqT", [DQ, LQ], BF16, kind="ExternalInput")
    xkvT = nc.dram_tensor("xkvT", [DKV, LKV], BF16, kind="ExternalInput")
    wq = nc.dram_tensor("wq", [DQ, DQ], BF16, kind="ExternalInput")
    wk = nc.dram_tensor("wk", [DKV, DQ], BF16, kind="ExternalInput")
    wv = nc.dram_tensor("wv", [DKV, DQ], BF16, kind="ExternalInput")
    wo = nc.dram_tensor("wo", [DQ, OD], BF16, kind="ExternalInput")
    bo = nc.dram_tensor("bo", [OD], F32, kind="ExternalInput")
    out = nc.dram_tensor("out", [LQ, OD], F32, kind="ExternalOutput")

    xq_r = xqT[:].rearrange("(dc p) l -> p dc l", p=128)
    xkv_r = xkvT[:].rearrange("(dc p) l -> p dc l", p=128)
    wq_r = wq[:].rearrange("(dc p) o -> p dc o", p=128)
    wk_r = wk[:].rearrange("(dc p) o -> p dc o", p=128)
    wv_r = wv[:].rearrange("(dc p) o -> p dc o", p=128)
    wo_r = wo[:].rearrange("(fc p) o -> p fc o", p=128)

    with tile.TileContext(nc) as tc:
        with (
            tc.tile_pool(name="persist", bufs=1) as persist,
            tc.tile_pool(name="xpool", bufs=1) as xpool,
            tc.tile_pool(name="wpool", bufs=2) as wpool,
            tc.tile_pool(name="qkpool", bufs=1) as qkpool,
            tc.tile_pool(name="vpool", bufs=4) as vpool,
            tc.tile_pool(name="epool", bufs=4) as epool,
            tc.tile_pool(name="rpool", bufs=4) as rpool,
            tc.tile_pool(name="opool", bufs=3) as opool,
            tc.tile_pool(name="mm_ps", bufs=2, space="PSUM") as mm_ps,
            tc.tile_pool(name="s_ps", bufs=2, space="PSUM") as s_ps,
            tc.tile_pool(name="o_ps", bufs=2, space="PSUM") as o_ps,
        ):
            # ---------- persistent tiles ----------
            # X tiles split so early projections can start before the full
            # input lands; spread across the three DMA-capable queues.
            XqTs = [xpool.tile([128, NDCQ, 512], BF16, name=f"xqt{i}")
                    for i in range(2)]
            XkvTs = [xpool.tile([128, NDCK, 1024], BF16, name=f"xkvt{i}")
                     for i in range(2)]
            nc.sync.dma_start(out=XkvTs[0][:], in_=xkv_r[:, :, 0:1024])
            nc.gpsimd.dma_start(out=XkvTs[1][:], in_=xkv_r[:, :, 1024:2048])

            def xq_slice(dc, lo, hi):
                t = lo // 512
                assert hi <= (t + 1) * 512
                return XqTs[t][:, dc, lo - t * 512:hi - t * 512]

            def xkv_slice(dc, lo, hi):
                t = lo // 1024
                assert hi <= (t + 1) * 1024
                return XkvTs[t][:, dc, lo - t * 1024:hi - t * 1024]

            bo_bcast = persist.tile([128, OD], F32)
            bo_ap = bass.AP(tensor=bo[:].tensor, offset=bo[:].offset,
                            ap=[[0, 128]] + list(bo[:].ap))
            nc.gpsimd.dma_start(out=bo_bcast[:], in_=bo_ap)

            wo_t = persist.tile([128, NDCQ, OD], BF16)

            qt = [persist.tile([128, LQ], BF16, name=f"qt{p}")
                  for p in range(NPAIR)]
            kt = [persist.tile([128, LKV], BF16, name=f"kt{p}")
                  for p in range(NPAIR)]
            OTn = [persist.tile([128, LQ], BF16, name=f"otn{p}")
                   for p in range(NPAIR)]
            V2 = {}

            # ---------- phase B generator: projections for pair p ----------
            def proj_pair(p):
                # Q^T pair cols [128, LQ]
                wq_t = wpool.tile([128, NDCQ, 128], BF16, tag="wq")
                nc.scalar.dma_start(out=wq_t[:],
                                    in_=wq_r[:, :, p * 128:(p + 1) * 128])
                if p == 0:
                    # input loads queued behind the first pair's weights so
                    # the first matmul can start as early as possible
                    nc.scalar.dma_start(out=XqTs[0][:], in_=xq_r[:, :, 0:512])
                    nc.sync.dma_start(out=XqTs[1][:], in_=xq_r[:, :, 512:1024])
                if p == 2:
                    # out-proj weights are only needed in phase D
                    nc.gpsimd.dma_start(out=wo_t[:], in_=wo_r)
                for qb in range(NQB):
                    pq = mm_ps.tile([128, 512], F32, tag="mm")
                    for dc in range(NDCQ):
                        nc.tensor.matmul(
                            pq[:], wq_t[:, dc, :],
                            xq_slice(dc, qb * 512, (qb + 1) * 512),
                            start=(dc == 0), stop=(dc == NDCQ - 1))
                        if dc % 4 == 1:
                            yield
                    nc.scalar.copy(qt[p][:, qb * 512:(qb + 1) * 512], pq[:])
                    yield

                # K^T pair cols [128, LKV]
                wk_t = wpool.tile([128, NDCK, 128], BF16, tag="wk")
                nc.scalar.dma_start(out=wk_t[:],
                                    in_=wk_r[:, :, p * 128:(p + 1) * 128])
                for kb in range(LKV // 512):
                    pk = mm_ps.tile([128, 512], F32, tag="mm")
                    for dc in range(NDCK):
                        nc.tensor.matmul(
                            pk[:], wk_t[:, dc, :],
                            xkv_slice(dc, kb * 512, (kb + 1) * 512),
                            start=(dc == 0), stop=(dc == NDCK - 1))
                        if dc % 3 == 1:
                            yield
                    nc.scalar.copy(kt[p][:, kb * 512:(kb + 1) * 512], pk[:])
                    yield

                # V natural [kv, 4 heads x 64] bf16 for the pair-GROUP
                # (built once per even pair, covers pairs p and p+1);
                # ones cols 64:96 per head for the softmax denominator.
                if p % 2 == 1:
                    return
                wv_t = wpool.tile([128, NDCK, 256], BF16, tag="wv")
                nc.scalar.dma_start(out=wv_t[:],
                                    in_=wv_r[:, :, p * 128:(p + 2) * 128])
                vts = []
                for x in range(2):
                    v2 = vpool.tile([128, NKC, 2, 96], BF16, tag="v2",
                                    name=f"v2_{p + x}")
                    V2[p + x] = v2
                    vts.append(v2)
                    nc.gpsimd.memset(v2[:, :, :, 64:96], 1.0)
                for kc2 in range(NKC // 2):
                    pv = mm_ps.tile([128, 512], F32, tag="mm")
                    for j in range(2):
                        kc = 2 * kc2 + j
                        for dc in range(NDCK):
                            nc.tensor.matmul(
                                pv[:, j * 256:(j + 1) * 256],
                                xkv_slice(dc, kc * 128, (kc + 1) * 128),
                                wv_t[:, dc, :],
                                start=(dc == 0), stop=(dc == NDCK - 1))
                        yield
                    pv5 = pv[:].rearrange("p (j pr h d) -> p j pr h d",
                                          j=2, pr=2, h=2)
                    for x in range(2):
                        nc.vector.tensor_copy(
                            vts[x][:, kc2 * 2:kc2 * 2 + 2, :, 0:64],
                            pv5[:, :, x, :, :])
                    yield

            # ---------- phase C generator: attention for pair p ----------
            def attn_pair(p, feeder):
                v2 = V2[p]
                ei = 0
                for qb in range(NQB):
                    po = [o_ps.tile([128, 512], F32, tag="o", name=f"po{h}")
                          for h in range(2)]

                    def do_pv(b, e2b):
                        # PV for iteration b (lagged one iter behind S so
                        # the PE never waits on the exp engines)
                        for j in range(2):
                            kc = 2 * b + j
                            st = (b == 0 and j == 0)
                            sp = (b == NB - 1 and j == 1)
                            for h in range(2):
                                nc.tensor.matmul(
                                    po[h][0:96, :], v2[:, kc, h, :],
                                    e2b[h][:, j * 512:(j + 1) * 512],
                                    start=st, stop=sp)

                    prev_e2 = None
                    for b in range(NB):
                        s2 = [s_ps.tile([128, 1024], F32, tag="s",
                                        name=f"s{h}") for h in range(2)]
                        e2 = [epool.tile([128, 1024], BF16, tag="e",
                                         name=f"e{h}") for h in range(2)]
                        for j in range(2):
                            kc = 2 * b + j
                            for h in range(2):
                                r0 = h * 64
                                nc.tensor.matmul(
                                    s2[h][:, j * 512:(j + 1) * 512],
                                    kt[p][r0:r0 + 64,
                                          kc * 128:(kc + 1) * 128],
                                    qt[p][r0:r0 + 64,
                                          qb * 512:(qb + 1) * 512],
                                    start=True, stop=True)
                        next(feeder, None)
                        for h in range(2):
                            if (ei + h) % 2 == 0:
                                nc.scalar.activation(e2[h][:], s2[h][:],
                                                     AF.Exp, scale=SCALE)
                            else:
                                nc.vector.tensor_scalar(
                                    e2[h][:].bitcast(U16), s2[h][:],
                                    A16 * SCALE, C16, OP.mult, OP.add)
                        ei += 1
                        if prev_e2 is not None:
                            do_pv(b - 1, prev_e2)
                        prev_e2 = e2
                        next(feeder, None)
                        yield
                    do_pv(NB - 1, prev_e2)
                    # normalize: rinv = bitcast-trick + 1 Newton step, then
                    # per-32-row-slab muls (DVE op cost ~ free dim only)
                    for h in range(2):
                        ri = rpool.tile([32, 512], F32, tag="ri",
                                        name=f"ri{h}")
                        rt = rpool.tile([32, 512], F32, tag="rt",
                                        name=f"rt{h}")
                        nc.vector.tensor_scalar(
                            ri[:].bitcast(I32), po[h][64:96, :].bitcast(I32),
                            -1, 0x7EEF127F, OP.mult, OP.add)
                        nc.vector.scalar_tensor_tensor(
                            rt[:], po[h][64:96, :], 1.0, ri[:],
                            OP.mult, OP.mult)
                        nc.vector.tensor_scalar(
                            rt[:], rt[:], -1.0, 2.0, OP.mult, OP.add)
                        nc.vector.tensor_tensor(
                            ri[:], ri[:], rt[:], OP.mult)
                        for s in range(2):
                            nc.vector.tensor_tensor(
                                OTn[p][h * 64 + s * 32:h * 64 + s * 32 + 32,
                                       qb * 512:(qb + 1) * 512],
                                po[h][s * 32:s * 32 + 32, :], ri[:], OP.mult)
                        next(feeder, None)
                    yield

            def run_all(gen):
                for _ in gen:
                    pass

            # ---------- drive pipeline ----------
            run_all(proj_pair(0))
            for p in range(NPAIR):
                feeder = proj_pair(p + 1) if p + 1 < NPAIR else iter(())
                for _ in attn_pair(p, feeder):
                    pass
                run_all(feeder)

            # ---------- phase D: out projection ----------
            for ob in range(OD // 512):
                for lc in range(LQ // 128):
                    pf = mm_ps.tile([128, 512], F32, tag="mm")
                    for fc in range(NDCQ):
                        nc.tensor.matmul(
                            pf[:], OTn[fc][:, lc * 128:(lc + 1) * 128],
                            wo_t[:, fc, ob * 512:(ob + 1) * 512],
                            start=(fc == 0), stop=(fc == NDCQ - 1))
                    o_sb = opool.tile([128, 512], F32, tag="osb")
                    nc.vector.tensor_tensor(
                        o_sb[:], pf[:], bo_bcast[:, ob * 512:(ob + 1) * 512],
                        OP.add)
                    nc.sync.dma_start(
                        out=out[lc * 128:(lc + 1) * 128,
                                ob * 512:(ob + 1) * 512],
                        in_=o_sb[:])

    return nc


_CACHED = {}


def get_nc():
    if "nc" not in _CACHED:
        nc = bacc.Bacc("TRN2", target_bir_lowering=False)
        build(nc)
        nc.finalize()
        _CACHED["nc"] = nc
    return _CACHED["nc"]


def make_in_maps(inputs):
    import ml_dtypes
    bf = ml_dtypes.bfloat16
    query = np.asarray(inputs["query"], dtype=np.float32)
    kv = np.asarray(inputs["kv"], dtype=np.float32)
    Wq = np.asarray(inputs["Wq"], dtype=np.float32).astype(bf)
    Wk = np.asarray(inputs["Wk"], dtype=np.float32).astype(bf)
    Wv = np.asarray(inputs["Wv"], dtype=np.float32).astype(bf)
    Wo = np.asarray(inputs["Wo"], dtype=np.float32).astype(bf)
    bo = np.asarray(inputs["bo"], dtype=np.float32)
    in_maps = []
    for c in range(8):
        b, hh = c // 2, c % 2
        xqT = np.ascontiguousarray(
            query[b, hh * LQ:(hh + 1) * LQ, :].T).astype(bf)
        xkvT = np.ascontiguousarray(kv[b].T).astype(bf)
        in_maps.append({
            "xqT": xqT, "xkvT": xkvT,
            "wq": Wq, "wk": Wk, "wv": Wv, "wo": Wo, "bo": bo,
        })
    return in_maps


def assemble_output(results):
    out_full = np.empty((4, 2048, OD), dtype=np.float32)
    for c in range(8):
        b, hh = c // 2, c % 2
        out_full[b, hh * LQ:(hh + 1) * LQ, :] = results[c]["out"]
    return out_full


def kernel(query, kv, Wq, Wk, Wv, Wo, bo, **run_kwargs):
    N, Lq_full, _ = np.asarray(query).shape
    assert (N, Lq_full) == (4, 2048)
    nc = get_nc()
    in_maps = make_in_maps(dict(query=query, kv=kv, Wq=Wq, Wk=Wk, Wv=Wv,
                                Wo=Wo, bo=bo))
    res = run_bass_kernel_spmd(nc, in_maps, core_ids=list(range(8)),
                               **run_kwargs)
    out_full = assemble_output(res.results)
    if run_kwargs:
        kernel.last_result = res
    return out_full


# revision 3
# speedup vs baseline: 1.1298x; 1.1298x over previous
"""TRN2 Bass kernel v2 for nn_CrossAttention (N=4, Lq=Lkv=2048, H=16, hd=64).

Sharding: 8 cores = (batch b = core//2) x (query-length half = core%2).
Each core: cross-attention for 1024 query rows of one batch, all 2048 kv.

Host-side prep: inputs are pre-transposed and cast to bf16 (xqT [DQ, LQ],
xkvT [DKV, LKV]) so no on-chip transpose phase is needed.

Pipeline (per core):
  B(p): projections for head-pair p (Q^T, K^T pair cols; V natural+ones)
  C(p): attention for pair p: per (qb, kc-pair): 4 S-matmuls (2-way row
        tiled, K=64 strips at partitions 0/64), exp split ACT(exact)/
        DVE(Schraudolph u16->bf16), 4 PV matmuls accumulating po[128,512]
        (rows 0:64 = O^T, rows 64:128 = softmax denom via ones cols).
        Normalize with DVE bitcast-reciprocal + Newton.
  B(p+1) matmuls are interleaved into C(p) so PE never starves while
  ACT/DVE chew on exp.
  D: out-projection OTn^T @ Wo + bias.
"""

import math
import os

import numpy as np

import concourse.bass as bass
import concourse.mybir as mybir
import concourse.tile as tile
from concourse import bacc
from concourse.bass_utils import run_bass_kernel_spmd

F32 = mybir.dt.float32
BF16 = mybir.dt.bfloat16
F8 = mybir.dt.float8e4
U16 = mybir.dt.uint16
I32 = mybir.dt.int32
AF = mybir.ActivationFunctionType
OP = mybir.AluOpType

DQ = 1024      # query feature dim
DKV = 768      # kv feature dim
LQ = 1024      # per-core query rows
LKV = 2048     # kv rows
H = 16         # heads
HD = 64        # head dim
OD = 1024      # output dim
NPAIR = 8      # head pairs (128 cols each)
SCALE = HD ** -0.5

NDCQ = DQ // 128    # 8 d-chunks for query features
NDCK = DKV // 128   # 6 d-chunks for kv features
NKC = LKV // 128    # 16 lkv chunks
NB = NKC // 2       # 8 kc-pairs per (pair, qb) iteration
NQB = LQ // 512     # 2 q blocks

# Schraudolph exp constants: u16 = round(A16*(SCALE*s) + C16), bitcast bf16
A16 = 128.0 / math.log(2.0)
C16 = 16250.65  # optimal ~16250.4 (round-nearest) / ~16250.9 (truncate)
# ACT share of exp ops out of 16 (rest go to DVE)
ACT_SHARE16 = int(os.environ.get("BASS_ACT_SHARE", "8"))


def build(nc: bass.Bass):
    xqT = nc.dram_tensor("xqT", [DQ, LQ], BF16, kind="ExternalInput")
    xkvT = nc.dram_tensor("xkvT", [DKV, LKV], BF16, kind="ExternalInput")
    wq = nc.dram_tensor("wq", [DQ, DQ], BF16, kind="ExternalInput")
    wk = nc.dram_tensor("wk", [DKV, DQ], BF16, kind="ExternalInput")
    wv = nc.dram_tensor("wv", [DKV, DQ], BF16, kind="ExternalInput")
    wo = nc.dram_tensor("wo", [DQ, OD], BF16, kind="ExternalInput")
    bo = nc.dram_tensor("bo", [OD], F32, kind="ExternalInput")
    out = nc.dram_tensor("out", [LQ, OD], F32, kind="ExternalOutput")

    xq_r = xqT[:].rearrange("(dc p) l -> p dc l", p=128)
    xkv_r = xkvT[:].rearrange("(dc p) l -> p dc l", p=128)
    wq_r = wq[:].rearrange("(dc p) o -> p dc o", p=128)
    wk_r = wk[:].rearrange("(dc p) o -> p dc o", p=128)
    wv_r = wv[:].rearrange("(dc p) o -> p dc o", p=128)
    wo_r = wo[:].rearrange("(fc p) o -> p fc o", p=128)

    with tile.TileContext(nc) as tc:
        with (
            tc.tile_pool(name="persist", bufs=1) as persist,
            tc.tile_pool(name="xpool", bufs=1) as xpool,
            tc.tile_pool(name="wpool", bufs=2) as wpool,
            tc.tile_pool(name="qkpool", bufs=1) as qkpool,
            tc.tile_pool(name="vpool", bufs=4) as vpool,
            tc.tile_pool(name="epool", bufs=4) as epool,
            tc.tile_pool(name="rpool", bufs=2) as rpool,
            tc.tile_pool(name="opool", bufs=3) as opool,
            tc.tile_pool(name="mm_ps", bufs=2, space="PSUM") as mm_ps,
            tc.tile_pool(name="s_ps", bufs=2, space="PSUM") as s_ps,
            tc.tile_pool(name="o_ps", bufs=2, space="PSUM") as o_ps,
        ):
            # ---------- persistent tiles ----------
            # X tiles split so early projections can start before the full
            # input lands; spread across the three DMA-capable queues.
            XqTs = [xpool.tile([128, NDCQ, 512], BF16, name=f"xqt{i}")
                    for i in range(2)]
            XkvTs = [xpool.tile([128, NDCK, 1024], BF16, name=f"xkvt{i}")
                     for i in range(2)]
            nc.sync.dma_start(out=XkvTs[0][:], in_=xkv_r[:, :, 0:1024])
            nc.gpsimd.dma_start(out=XkvTs[1][:], in_=xkv_r[:, :, 1024:2048])

            def xq_slice(dc, lo, hi):
                t = lo // 512
                assert hi <= (t + 1) * 512
                return XqTs[t][:, dc, lo - t * 512:hi - t * 512]

            def xkv_slice(dc, lo, hi):
                t = lo // 1024
                assert hi <= (t + 1) * 1024
                return XkvTs[t][:, dc, lo - t * 1024:hi - t * 1024]

            bo_bcast = persist.tile([128, OD], F32)
            bo_ap = bass.AP(tensor=bo[:].tensor, offset=bo[:].offset,
                            ap=[[0, 128]] + list(bo[:].ap))
            nc.gpsimd.dma_start(out=bo_bcast[:], in_=bo_ap)

            wo_t = persist.tile([128, NDCQ, OD], BF16)

            qt = [persist.tile([128, LQ], BF16, name=f"qt{p}")
                  for p in range(NPAIR)]
            kt = [persist.tile([128, LKV], BF16, name=f"kt{p}")
                  for p in range(NPAIR)]
            OTn = [persist.tile([128, LQ], BF16, name=f"otn{p}")
                   for p in range(NPAIR)]
            V2 = {}

            # ---------- phase B generator: projections for pair p ----------
            def proj_pair(p):
                # Q^T pair cols [128, LQ]
                wq_t = wpool.tile([128, NDCQ, 128], BF16, tag="wq")
                nc.scalar.dma_start(out=wq_t[:],
                                    in_=wq_r[:, :, p * 128:(p + 1) * 128])
                if p == 0:
                    # input loads queued behind the first pair's weights so
                    # the first matmul can start as early as possible; the
                    # first XqT half is split across two queues
                    nc.scalar.dma_start(out=XqTs[0][:, 0:4, :],
                                        in_=xq_r[:, 0:4, 0:512])
                    nc.sync.dma_start(out=XqTs[0][:, 4:8, :],
                                      in_=xq_r[:, 4:8, 0:512])
                    nc.scalar.dma_start(out=XqTs[1][:], in_=xq_r[:, :, 512:1024])
                if p == 2:
                    # out-proj weights are only needed in phase D
                    nc.gpsimd.dma_start(out=wo_t[:], in_=wo_r)
                for qb in range(NQB):
                    pq = mm_ps.tile([128, 512], F32, tag="mm")
                    for dc in range(NDCQ):
                        nc.tensor.matmul(
                            pq[:], wq_t[:, dc, :],
                            xq_slice(dc, qb * 512, (qb + 1) * 512),
                            start=(dc == 0), stop=(dc == NDCQ - 1))
                        if dc % 4 == 1:
                            yield
                    nc.scalar.copy(qt[p][:, qb * 512:(qb + 1) * 512], pq[:])
                    yield

                # K^T pair cols [128, LKV]
                wk_t = wpool.tile([128, NDCK, 128], BF16, tag="wk")
                nc.scalar.dma_start(out=wk_t[:],
                                    in_=wk_r[:, :, p * 128:(p + 1) * 128])
                for kb in range(LKV // 512):
                    pk = mm_ps.tile([128, 512], F32, tag="mm")
                    for dc in range(NDCK):
                        nc.tensor.matmul(
                            pk[:], wk_t[:, dc, :],
                            xkv_slice(dc, kb * 512, (kb + 1) * 512),
                            start=(dc == 0), stop=(dc == NDCK - 1))
                        if dc % 3 == 1:
                            yield
                    nc.scalar.copy(kt[p][:, kb * 512:(kb + 1) * 512], pk[:])
                    yield

                # V natural [kv, 4 heads x 64] bf16 for the pair-GROUP
                # (built once per even pair, covers pairs p and p+1);
                # ones cols 64:96 per head for the softmax denominator.
                if p % 2 == 1:
                    return
                wv_t = wpool.tile([128, NDCK, 256], BF16, tag="wv")
                nc.scalar.dma_start(out=wv_t[:],
                                    in_=wv_r[:, :, p * 128:(p + 2) * 128])
                vts = []
                for x in range(2):
                    v2 = vpool.tile([128, NKC, 2, 128], BF16, tag="v2",
                                    name=f"v2_{p + x}")
                    V2[p + x] = v2
                    vts.append(v2)
                    nc.gpsimd.memset(v2[:, :, :, 64:128], 1.0)
                for kc2 in range(NKC // 2):
                    pv = mm_ps.tile([128, 512], F32, tag="mm")
                    for j in range(2):
                        kc = 2 * kc2 + j
                        for dc in range(NDCK):
                            nc.tensor.matmul(
                                pv[:, j * 256:(j + 1) * 256],
                                xkv_slice(dc, kc * 128, (kc + 1) * 128),
                                wv_t[:, dc, :],
                                start=(dc == 0), stop=(dc == NDCK - 1))
                        yield
                    pv5 = pv[:].rearrange("p (j pr h d) -> p j pr h d",
                                          j=2, pr=2, h=2)
                    for x in range(2):
                        nc.vector.tensor_copy(
                            vts[x][:, kc2 * 2:kc2 * 2 + 2, :, 0:64],
                            pv5[:, :, x, :, :])
                    yield

            # ---------- phase C generator: attention for pair p ----------
            def attn_pair(p, feeder):
                v2 = V2[p]
                ei = 0
                for qb in range(NQB):
                    po = [o_ps.tile([128, 512], F32, tag="o", name=f"po{h}")
                          for h in range(2)]

                    def do_pv(b, e2b):
                        # PV for iteration b (lagged one iter behind S so
                        # the PE never waits on the exp engines)
                        for j in range(2):
                            kc = 2 * b + j
                            st = (b == 0 and j == 0)
                            sp = (b == NB - 1 and j == 1)
                            for h in range(2):
                                nc.tensor.matmul(
                                    po[h][:], v2[:, kc, h, :],
                                    e2b[h][:, j * 512:(j + 1) * 512],
                                    start=st, stop=sp)

                    prev_e2 = None
                    for b in range(NB):
                        s2 = [s_ps.tile([128, 1024], F32, tag="s",
                                        name=f"s{h}") for h in range(2)]
                        e2 = [epool.tile([128, 1024], BF16, tag="e",
                                         name=f"e{h}") for h in range(2)]
                        for j in range(2):
                            kc = 2 * b + j
                            for h in range(2):
                                r0 = h * 64
                                nc.tensor.matmul(
                                    s2[h][:, j * 512:(j + 1) * 512],
                                    kt[p][r0:r0 + 64,
                                          kc * 128:(kc + 1) * 128],
                                    qt[p][r0:r0 + 64,
                                          qb * 512:(qb + 1) * 512],
                                    start=True, stop=True)
                        next(feeder, None)
                        for h in range(2):
                            if (ei + h) % 2 == 0:
                                nc.scalar.activation(e2[h][:], s2[h][:],
                                                     AF.Exp, scale=SCALE)
                            else:
                                nc.vector.tensor_scalar(
                                    e2[h][:].bitcast(U16), s2[h][:],
                                    A16 * SCALE, C16, OP.mult, OP.add)
                        ei += 1
                        if prev_e2 is not None:
                            do_pv(b - 1, prev_e2)
                        prev_e2 = e2
                        next(feeder, None)
                        yield
                    do_pv(NB - 1, prev_e2)
                    # evacuate po to SBUF fast (2 ACT copies per head,
                    # PSUM source so partition offsets may differ) so the
                    # PSUM bank frees quickly; the normalize chain
                    # (bitcast-reciprocal + Newton + mul) then runs on DVE
                    # off the PE critical path on partition-aligned tiles.
                    for h in range(2):
                        ocO = rpool.tile([64, 512], F32, tag="ocO",
                                         name=f"ocO{h}")
                        ocR = rpool.tile([64, 512], F32, tag="ocR",
                                         name=f"ocR{h}")
                        nc.scalar.copy(ocO[:], po[h][0:64, :])
                        nc.scalar.copy(ocR[:], po[h][64:128, :])
                        ri = rpool.tile([64, 512], F32, tag="ri",
                                        name=f"ri{h}")
                        rt = rpool.tile([64, 512], F32, tag="rt",
                                        name=f"rt{h}")
                        nc.vector.tensor_scalar(
                            ri[:].bitcast(I32), ocR[:].bitcast(I32),
                            -1, 0x7EEF127F, OP.mult, OP.add)
                        nc.vector.scalar_tensor_tensor(
                            rt[:], ocR[:], 1.0, ri[:], OP.mult, OP.mult)
                        nc.vector.tensor_scalar(
                            rt[:], rt[:], -1.0, 2.0, OP.mult, OP.add)
                        nc.vector.tensor_tensor(
                            ri[:], ri[:], rt[:], OP.mult)
                        nc.vector.tensor_tensor(
                            OTn[p][h * 64:(h + 1) * 64,
                                   qb * 512:(qb + 1) * 512],
                            ocO[:], ri[:], OP.mult)
                        next(feeder, None)
                    yield

            def run_all(gen):
                for _ in gen:
                    pass

            # ---------- drive pipeline ----------
            run_all(proj_pair(0))
            for p in range(NPAIR):
                feeder = proj_pair(p + 1) if p + 1 < NPAIR else iter(())
                for _ in attn_pair(p, feeder):
                    pass
                run_all(feeder)

            # ---------- phase D: out projection ----------
            for ob in range(OD // 512):
                for lc in range(LQ // 128):
                    pf = mm_ps.tile([128, 512], F32, tag="mm")
                    for fc in range(NDCQ):
                        nc.tensor.matmul(
                            pf[:], OTn[fc][:, lc * 128:(lc + 1) * 128],
                            wo_t[:, fc, ob * 512:(ob + 1) * 512],
                            start=(fc == 0), stop=(fc == NDCQ - 1))
                    o_sb = opool.tile([128, 512], F32, tag="osb")
                    nc.vector.tensor_tensor(
                        o_sb[:], pf[:], bo_bcast[:, ob * 512:(ob + 1) * 512],
                        OP.add)
                    q = [nc.sync, nc.scalar, nc.gpsimd][lc % 3]
                    q.dma_start(
                        out=out[lc * 128:(lc + 1) * 128,
                                ob * 512:(ob + 1) * 512],
                        in_=o_sb[:])

    return nc


_CACHED = {}


def get_nc():
    if "nc" not in _CACHED:
        nc = bacc.Bacc("TRN2", target_bir_lowering=False)
        build(nc)
        nc.finalize()
        _CACHED["nc"] = nc
    return _CACHED["nc"]


def make_in_maps(inputs):
    import ml_dtypes
    bf = ml_dtypes.bfloat16
    query = np.asarray(inputs["query"], dtype=np.float32)
    kv = np.asarray(inputs["kv"], dtype=np.float32)
    Wq = np.asarray(inputs["Wq"], dtype=np.float32).astype(bf)
    Wk = np.asarray(inputs["Wk"], dtype=np.float32).astype(bf)
    Wv = np.asarray(inputs["Wv"], dtype=np.float32).astype(bf)
    Wo = np.asarray(inputs["Wo"], dtype=np.float32).astype(bf)
    bo = np.asarray(inputs["bo"], dtype=np.float32)
    in_maps = []
    for c in range(8):
        b, hh = c // 2, c % 2
        xqT = np.ascontiguousarray(
            query[b, hh * LQ:(hh + 1) * LQ, :].T).astype(bf)
        xkvT = np.ascontiguousarray(kv[b].T).astype(bf)
        in_maps.append({
            "xqT": xqT, "xkvT": xkvT,
            "wq": Wq, "wk": Wk, "wv": Wv, "wo": Wo, "bo": bo,
        })
    return in_maps


def assemble_output(results):
    out_full = np.empty((4, 2048, OD), dtype=np.float32)
    for c in range(8):
        b, hh = c // 2, c % 2
        out_full[b, hh * LQ:(hh + 1) * LQ, :] = results[c]["out"]
    return out_full


def kernel(query, kv, Wq, Wk, Wv, Wo, bo, **run_kwargs):
    N, Lq_full, _ = np.asarray(query).shape
    assert (N, Lq_full) == (4, 2048)
    nc = get_nc()
    in_maps = make_in_maps(dict(query=query, kv=kv, Wq=Wq, Wk=Wk, Wv=Wv,
                                Wo=Wo, bo=bo))
    res = run_bass_kernel_spmd(nc, in_maps, core_ids=list(range(8)),
                               **run_kwargs)
    out_full = assemble_output(res.results)
    if run_kwargs:
        kernel.last_result = res
    return out_full


# revision 4
# speedup vs baseline: 1.1476x; 1.0157x over previous
"""TRN2 Bass kernel v2 for nn_CrossAttention (N=4, Lq=Lkv=2048, H=16, hd=64).

Sharding: 8 cores = (batch b = core//2) x (query-length half = core%2).
Each core: cross-attention for 1024 query rows of one batch, all 2048 kv.

Host-side prep: inputs are pre-transposed and cast to bf16 (xqT [DQ, LQ],
xkvT [DKV, LKV]) so no on-chip transpose phase is needed.

Pipeline (per core):
  B(p): projections for head-pair p (Q^T, K^T pair cols; V natural+ones)
  C(p): attention for pair p: per (qb, kc-pair): 4 S-matmuls (2-way row
        tiled, K=64 strips at partitions 0/64), exp split ACT(exact)/
        DVE(Schraudolph u16->bf16), 4 PV matmuls accumulating po[128,512]
        (rows 0:64 = O^T, rows 64:128 = softmax denom via ones cols).
        Normalize with DVE bitcast-reciprocal + Newton.
  B(p+1) matmuls are interleaved into C(p) so PE never starves while
  ACT/DVE chew on exp.
  D: out-projection OTn^T @ Wo + bias.
"""

import math
import os

import numpy as np

import concourse.bass as bass
import concourse.mybir as mybir
import concourse.tile as tile
from concourse import bacc
from concourse.bass_utils import run_bass_kernel_spmd

F32 = mybir.dt.float32
BF16 = mybir.dt.bfloat16
F8 = mybir.dt.float8e4
U16 = mybir.dt.uint16
I32 = mybir.dt.int32
AF = mybir.ActivationFunctionType
OP = mybir.AluOpType

DQ = 1024      # query feature dim
DKV = 768      # kv feature dim
LQ = 1024      # per-core query rows
LKV = 2048     # kv rows
H = 16         # heads
HD = 64        # head dim
OD = 1024      # output dim
NPAIR = 8      # head pairs (128 cols each)
SCALE = HD ** -0.5

NDCQ = DQ // 128    # 8 d-chunks for query features
NDCK = DKV // 128   # 6 d-chunks for kv features
NKC = LKV // 128    # 16 lkv chunks
NB = NKC // 2       # 8 kc-pairs per (pair, qb) iteration
NQB = LQ // 512     # 2 q blocks

# Schraudolph exp constants: u16 = round(A16*(SCALE*s) + C16), bitcast bf16
A16 = 128.0 / math.log(2.0)
C16 = 16250.65  # optimal ~16250.4 (round-nearest) / ~16250.9 (truncate)
# ACT share of exp ops out of 16 (rest go to DVE)
ACT_SHARE16 = int(os.environ.get("BASS_ACT_SHARE", "8"))


def build(nc: bass.Bass):
    xqT = nc.dram_tensor("xqT", [DQ, LQ], BF16, kind="ExternalInput")
    xkvT = nc.dram_tensor("xkvT", [DKV, LKV], BF16, kind="ExternalInput")
    wq = nc.dram_tensor("wq", [DQ, DQ], BF16, kind="ExternalInput")
    wk = nc.dram_tensor("wk", [DKV, DQ], BF16, kind="ExternalInput")
    wv = nc.dram_tensor("wv", [DKV, DQ], BF16, kind="ExternalInput")
    wo = nc.dram_tensor("wo", [DQ, OD], BF16, kind="ExternalInput")
    bo = nc.dram_tensor("bo", [OD], F32, kind="ExternalInput")
    out = nc.dram_tensor("out", [LQ, OD], F32, kind="ExternalOutput")

    xq_r = xqT[:].rearrange("(dc p) l -> p dc l", p=128)
    xkv_r = xkvT[:].rearrange("(dc p) l -> p dc l", p=128)
    wq_r = wq[:].rearrange("(dc p) o -> p dc o", p=128)
    wk_r = wk[:].rearrange("(dc p) o -> p dc o", p=128)
    wv_r = wv[:].rearrange("(dc p) o -> p dc o", p=128)
    wo_r = wo[:].rearrange("(fc p) o -> p fc o", p=128)

    with tile.TileContext(nc) as tc:
        with (
            tc.tile_pool(name="persist", bufs=1) as persist,
            tc.tile_pool(name="xpool", bufs=1) as xpool,
            tc.tile_pool(name="wpool", bufs=2) as wpool,
            tc.tile_pool(name="qkpool", bufs=1) as qkpool,
            tc.tile_pool(name="vpool", bufs=4) as vpool,
            tc.tile_pool(name="epool", bufs=4) as epool,
            tc.tile_pool(name="rpool", bufs=2) as rpool,
            tc.tile_pool(name="opool", bufs=3) as opool,
            tc.tile_pool(name="mm_ps", bufs=2, space="PSUM") as mm_ps,
            tc.tile_pool(name="s_ps", bufs=2, space="PSUM") as s_ps,
            tc.tile_pool(name="o_ps", bufs=2, space="PSUM") as o_ps,
        ):
            # ---------- persistent tiles ----------
            # X tiles split so early projections can start before the full
            # input lands; spread across the three DMA-capable queues.
            XqTs = [xpool.tile([128, NDCQ, 512], BF16, name=f"xqt{i}")
                    for i in range(2)]
            XkvTs = [xpool.tile([128, NDCK, 1024], BF16, name=f"xkvt{i}")
                     for i in range(2)]
            nc.sync.dma_start(out=XkvTs[0][:], in_=xkv_r[:, :, 0:1024])
            nc.gpsimd.dma_start(out=XkvTs[1][:], in_=xkv_r[:, :, 1024:2048])

            def xq_slice(dc, lo, hi):
                t = lo // 512
                assert hi <= (t + 1) * 512
                return XqTs[t][:, dc, lo - t * 512:hi - t * 512]

            def xkv_slice(dc, lo, hi):
                t = lo // 1024
                assert hi <= (t + 1) * 1024
                return XkvTs[t][:, dc, lo - t * 1024:hi - t * 1024]

            bo_bcast = persist.tile([128, OD], F32)
            bo_ap = bass.AP(tensor=bo[:].tensor, offset=bo[:].offset,
                            ap=[[0, 128]] + list(bo[:].ap))
            nc.gpsimd.dma_start(out=bo_bcast[:], in_=bo_ap)

            wo_t = persist.tile([128, NDCQ, OD], BF16)

            qt = [persist.tile([128, LQ], BF16, name=f"qt{p}")
                  for p in range(NPAIR)]
            kt = [persist.tile([128, LKV], BF16, name=f"kt{p}")
                  for p in range(NPAIR)]
            OTn = [persist.tile([128, LQ], BF16, name=f"otn{p}")
                   for p in range(NPAIR)]
            V2 = {}

            # ---------- phase B generator: projections for pair p ----------
            def proj_pair(p):
                # Q^T pair cols [128, LQ]
                wq_t = wpool.tile([128, NDCQ, 128], BF16, tag="wq")
                nc.scalar.dma_start(out=wq_t[:],
                                    in_=wq_r[:, :, p * 128:(p + 1) * 128])
                if p == 0:
                    # input loads queued behind the first pair's weights so
                    # the first matmul can start as early as possible
                    nc.scalar.dma_start(out=XqTs[0][:], in_=xq_r[:, :, 0:512])
                    nc.sync.dma_start(out=XqTs[1][:], in_=xq_r[:, :, 512:1024])
                if p == 2:
                    # out-proj weights are only needed in phase D
                    nc.gpsimd.dma_start(out=wo_t[:], in_=wo_r)
                for qb in range(NQB):
                    pq = mm_ps.tile([128, 512], F32, tag="mm")
                    for dc in range(NDCQ):
                        nc.tensor.matmul(
                            pq[:], wq_t[:, dc, :],
                            xq_slice(dc, qb * 512, (qb + 1) * 512),
                            start=(dc == 0), stop=(dc == NDCQ - 1))
                        if dc % 4 == 1:
                            yield
                    nc.scalar.copy(qt[p][:, qb * 512:(qb + 1) * 512], pq[:])
                    yield

                # K^T pair cols [128, LKV]
                wk_t = wpool.tile([128, NDCK, 128], BF16, tag="wk")
                nc.scalar.dma_start(out=wk_t[:],
                                    in_=wk_r[:, :, p * 128:(p + 1) * 128])
                for kb in range(LKV // 512):
                    pk = mm_ps.tile([128, 512], F32, tag="mm")
                    for dc in range(NDCK):
                        nc.tensor.matmul(
                            pk[:], wk_t[:, dc, :],
                            xkv_slice(dc, kb * 512, (kb + 1) * 512),
                            start=(dc == 0), stop=(dc == NDCK - 1))
                        if dc % 3 == 1:
                            yield
                    nc.scalar.copy(kt[p][:, kb * 512:(kb + 1) * 512], pk[:])
                    yield

                # V natural [kv, 4 heads x 64] bf16 for the pair-GROUP
                # (built once per even pair, covers pairs p and p+1);
                # ones cols 64:96 per head for the softmax denominator.
                if p % 2 == 1:
                    return
                wv_t = wpool.tile([128, NDCK, 256], BF16, tag="wv")
                nc.scalar.dma_start(out=wv_t[:],
                                    in_=wv_r[:, :, p * 128:(p + 2) * 128])
                vts = []
                for x in range(2):
                    v2 = vpool.tile([128, NKC, 2, 128], BF16, tag="v2",
                                    name=f"v2_{p + x}")
                    V2[p + x] = v2
                    vts.append(v2)
                    nc.gpsimd.memset(v2[:, :, :, 64:128], 1.0)
                for kc2 in range(NKC // 2):
                    pv = mm_ps.tile([128, 512], F32, tag="mm")
                    for j in range(2):
                        kc = 2 * kc2 + j
                        for dc in range(NDCK):
                            nc.tensor.matmul(
                                pv[:, j * 256:(j + 1) * 256],
                                xkv_slice(dc, kc * 128, (kc + 1) * 128),
                                wv_t[:, dc, :],
                                start=(dc == 0), stop=(dc == NDCK - 1))
                        yield
                    pv5 = pv[:].rearrange("p (j pr h d) -> p j pr h d",
                                          j=2, pr=2, h=2)
                    for x in range(2):
                        nc.vector.tensor_copy(
                            vts[x][:, kc2 * 2:kc2 * 2 + 2, :, 0:64],
                            pv5[:, :, x, :, :])
                    yield

            # ---------- phase C generator: attention for pair p ----------
            def attn_pair(p, feeder):
                v2 = V2[p]
                ei = 0
                for qb in range(NQB):
                    po = [o_ps.tile([128, 512], F32, tag="o", name=f"po{h}")
                          for h in range(2)]

                    def do_pv(b, e2b):
                        # PV for iteration b (lagged one iter behind S so
                        # the PE never waits on the exp engines)
                        for j in range(2):
                            kc = 2 * b + j
                            st = (b == 0 and j == 0)
                            sp = (b == NB - 1 and j == 1)
                            for h in range(2):
                                nc.tensor.matmul(
                                    po[h][:], v2[:, kc, h, :],
                                    e2b[h][:, j * 512:(j + 1) * 512],
                                    start=st, stop=sp)

                    prev_e2 = None
                    for b in range(NB):
                        s2 = [s_ps.tile([128, 1024], F32, tag="s",
                                        name=f"s{h}") for h in range(2)]
                        e2 = [epool.tile([128, 1024], BF16, tag="e",
                                         name=f"e{h}") for h in range(2)]
                        for j in range(2):
                            kc = 2 * b + j
                            for h in range(2):
                                r0 = h * 64
                                nc.tensor.matmul(
                                    s2[h][:, j * 512:(j + 1) * 512],
                                    kt[p][r0:r0 + 64,
                                          kc * 128:(kc + 1) * 128],
                                    qt[p][r0:r0 + 64,
                                          qb * 512:(qb + 1) * 512],
                                    start=True, stop=True)
                        next(feeder, None)
                        for h in range(2):
                            if (ei + h) % 2 == 0:
                                nc.scalar.activation(e2[h][:], s2[h][:],
                                                     AF.Exp, scale=SCALE)
                            else:
                                nc.vector.tensor_scalar(
                                    e2[h][:].bitcast(U16), s2[h][:],
                                    A16 * SCALE, C16, OP.mult, OP.add)
                        ei += 1
                        if prev_e2 is not None:
                            do_pv(b - 1, prev_e2)
                        prev_e2 = e2
                        next(feeder, None)
                        yield
                    do_pv(NB - 1, prev_e2)
                    # evacuate po to SBUF fast (2 ACT copies per head,
                    # PSUM source so partition offsets may differ) so the
                    # PSUM bank frees quickly; the normalize chain
                    # (bitcast-reciprocal + Newton + mul) then runs on DVE
                    # off the PE critical path on partition-aligned tiles.
                    for h in range(2):
                        ocO = rpool.tile([64, 512], F32, tag="ocO",
                                         name=f"ocO{h}")
                        nc.scalar.copy(ocO[:], po[h][0:64, :])
                        ri = rpool.tile([64, 512], F32, tag="ri",
                                        name=f"ri{h}")
                        # rinv = exp(-ln(r)) on ACT (exp+ln share one table
                        # set, see get_nc); reads r straight from PSUM.
                        nc.scalar.activation(ri[:], po[h][64:128, :], AF.Ln)
                        nc.scalar.activation(ri[:], ri[:], AF.Exp,
                                             scale=-1.0)
                        nc.vector.tensor_tensor(
                            OTn[p][h * 64:(h + 1) * 64,
                                   qb * 512:(qb + 1) * 512],
                            ocO[:], ri[:], OP.mult)
                        next(feeder, None)
                    yield

            def run_all(gen):
                for _ in gen:
                    pass

            # ---------- phase D generator: out projection ----------
            def d_phase(lcs):
                for ob in range(OD // 512):
                    for lc in lcs:
                        pf = mm_ps.tile([128, 512], F32, tag="mm")
                        for fc in range(NDCQ):
                            nc.tensor.matmul(
                                pf[:], OTn[fc][:, lc * 128:(lc + 1) * 128],
                                wo_t[:, fc, ob * 512:(ob + 1) * 512],
                                start=(fc == 0), stop=(fc == NDCQ - 1))
                        o_sb = opool.tile([128, 512], F32, tag="osb")
                        nc.vector.tensor_tensor(
                            o_sb[:], pf[:],
                            bo_bcast[:, ob * 512:(ob + 1) * 512], OP.add)
                        q = [nc.sync, nc.scalar, nc.gpsimd][lc % 3]
                        q.dma_start(
                            out=out[lc * 128:(lc + 1) * 128,
                                    ob * 512:(ob + 1) * 512],
                            in_=o_sb[:])
                        yield

            # ---------- drive pipeline ----------
            run_all(proj_pair(0))
            for p in range(NPAIR):
                if p + 1 < NPAIR:
                    feeder = proj_pair(p + 1)
                else:
                    # last pair: q rows 0:512 of the out-projection only
                    # need qb=0 results; 18 null steps hold them out of the
                    # PE queue until the qb=1 loop (else the queued D MMs
                    # would stall the PE behind unmet OTn[7] deps)
                    import itertools
                    feeder = itertools.chain(
                        iter([None] * 18), d_phase([0, 1, 2, 3]))
                for _ in attn_pair(p, feeder):
                    pass
                run_all(feeder)
            run_all(d_phase([4, 5, 6, 7]))

    return nc


_CACHED = {}


def _force_combined_exp_ln_table(arch):
    # The act-table placement pass picks, per activation, some set
    # containing its function; with both Exp and Ln in use it ping-pongs
    # between 'exp_and_others' and 'natural_log' (a ~2.7us reload each
    # time). Trim the cached table map (indices preserved - they are the
    # act_func_set_id) so Exp and Ln only resolve to the combined
    # 'natural_log_exp_and_others' set.
    import concourse.hw_specs as hw_specs
    tabs = hw_specs.get_activation_tables(arch)
    exp_t = mybir.ActivationFunctionType.Exp
    ln_t = mybir.ActivationFunctionType.Ln
    for name, fns in tabs.items():
        if name != "natural_log_exp_and_others":
            fns.discard(exp_t)
            fns.discard(ln_t)


def get_nc():
    if "nc" not in _CACHED:
        nc = bacc.Bacc("TRN2", target_bir_lowering=False)
        build(nc)
        _force_combined_exp_ln_table(nc.m.arch)
        nc.finalize()
        _CACHED["nc"] = nc
    return _CACHED["nc"]


def make_in_maps(inputs):
    import ml_dtypes
    bf = ml_dtypes.bfloat16
    query = np.asarray(inputs["query"], dtype=np.float32)
    kv = np.asarray(inputs["kv"], dtype=np.float32)
    Wq = np.asarray(inputs["Wq"], dtype=np.float32).astype(bf)
    Wk = np.asarray(inputs["Wk"], dtype=np.float32).astype(bf)
    Wv = np.asarray(inputs["Wv"], dtype=np.float32).astype(bf)
    Wo = np.asarray(inputs["Wo"], dtype=np.float32).astype(bf)
    bo = np.asarray(inputs["bo"], dtype=np.float32)
    in_maps = []
    for c in range(8):
        b, hh = c // 2, c % 2
        xqT = np.ascontiguousarray(
            query[b, hh * LQ:(hh + 1) * LQ, :].T).astype(bf)
        xkvT = np.ascontiguousarray(kv[b].T).astype(bf)
        in_maps.append({
            "xqT": xqT, "xkvT": xkvT,
            "wq": Wq, "wk": Wk, "wv": Wv, "wo": Wo, "bo": bo,
        })
    return in_maps


def assemble_output(results):
    out_full = np.empty((4, 2048, OD), dtype=np.float32)
    for c in range(8):
        b, hh = c // 2, c % 2
        out_full[b, hh * LQ:(hh + 1) * LQ, :] = results[c]["out"]
    return out_full


def kernel(query, kv, Wq, Wk, Wv, Wo, bo, **run_kwargs):
    N, Lq_full, _ = np.asarray(query).shape
    assert (N, Lq_full) == (4, 2048)
    nc = get_nc()
    in_maps = make_in_maps(dict(query=query, kv=kv, Wq=Wq, Wk=Wk, Wv=Wv,
                                Wo=Wo, bo=bo))
    res = run_bass_kernel_spmd(nc, in_maps, core_ids=list(range(8)),
                               **run_kwargs)
    out_full = assemble_output(res.results)
    if run_kwargs:
        kernel.last_result = res
    return out_full


# revision 5
# speedup vs baseline: 1.1498x; 1.0019x over previous
"""TRN2 Bass kernel v2 for nn_CrossAttention (N=4, Lq=Lkv=2048, H=16, hd=64).

Sharding: 8 cores = (batch b = core//2) x (query-length half = core%2).
Each core: cross-attention for 1024 query rows of one batch, all 2048 kv.

Host-side prep: inputs are pre-transposed and cast to bf16 (xqT [DQ, LQ],
xkvT [DKV, LKV]) so no on-chip transpose phase is needed.

Pipeline (per core):
  B(p): projections for head-pair p (Q^T, K^T pair cols; V natural+ones)
  C(p): attention for pair p: per (qb, kc-pair): 4 S-matmuls (2-way row
        tiled, K=64 strips at partitions 0/64), exp split ACT(exact)/
        DVE(Schraudolph u16->bf16), 4 PV matmuls accumulating po[128,512]
        (rows 0:64 = O^T, rows 64:128 = softmax denom via ones cols).
        Normalize with DVE bitcast-reciprocal + Newton.
  B(p+1) matmuls are interleaved into C(p) so PE never starves while
  ACT/DVE chew on exp.
  D: out-projection OTn^T @ Wo + bias.
"""

import math
import os

import numpy as np

import concourse.bass as bass
import concourse.mybir as mybir
import concourse.tile as tile
from concourse import bacc
from concourse.bass_utils import run_bass_kernel_spmd

F32 = mybir.dt.float32
BF16 = mybir.dt.bfloat16
F8 = mybir.dt.float8e4
U16 = mybir.dt.uint16
I32 = mybir.dt.int32
AF = mybir.ActivationFunctionType
OP = mybir.AluOpType

DQ = 1024      # query feature dim
DKV = 768      # kv feature dim
LQ = 1024      # per-core query rows
LKV = 2048     # kv rows
H = 16         # heads
HD = 64        # head dim
OD = 1024      # output dim
NPAIR = 8      # head pairs (128 cols each)
SCALE = HD ** -0.5

NDCQ = DQ // 128    # 8 d-chunks for query features
NDCK = DKV // 128   # 6 d-chunks for kv features
NKC = LKV // 128    # 16 lkv chunks
NB = NKC // 2       # 8 kc-pairs per (pair, qb) iteration
NQB = LQ // 512     # 2 q blocks

# Schraudolph exp constants: u16 = round(A16*(SCALE*s) + C16), bitcast bf16
A16 = 128.0 / math.log(2.0)
C16 = 16250.65  # optimal ~16250.4 (round-nearest) / ~16250.9 (truncate)
# ACT share of exp ops out of 16 (rest go to DVE)
ACT_SHARE16 = int(os.environ.get("BASS_ACT_SHARE", "8"))


def build(nc: bass.Bass):
    xqT = nc.dram_tensor("xqT", [DQ, LQ], BF16, kind="ExternalInput")
    xkvT = nc.dram_tensor("xkvT", [DKV, LKV], BF16, kind="ExternalInput")
    wq = nc.dram_tensor("wq", [DQ, DQ], BF16, kind="ExternalInput")
    wk = nc.dram_tensor("wk", [DKV, DQ], BF16, kind="ExternalInput")
    wv = nc.dram_tensor("wv", [DKV, DQ], BF16, kind="ExternalInput")
    wo = nc.dram_tensor("wo", [DQ, OD], BF16, kind="ExternalInput")
    bo = nc.dram_tensor("bo", [OD], F32, kind="ExternalInput")
    out = nc.dram_tensor("out", [LQ, OD], F32, kind="ExternalOutput")

    xq_r = xqT[:].rearrange("(dc p) l -> p dc l", p=128)
    xkv_r = xkvT[:].rearrange("(dc p) l -> p dc l", p=128)
    wq_r = wq[:].rearrange("(dc p) o -> p dc o", p=128)
    wk_r = wk[:].rearrange("(dc p) o -> p dc o", p=128)
    wv_r = wv[:].rearrange("(dc p) o -> p dc o", p=128)
    wo_r = wo[:].rearrange("(fc p) o -> p fc o", p=128)

    with tile.TileContext(nc) as tc:
        with (
            tc.tile_pool(name="persist", bufs=1) as persist,
            tc.tile_pool(name="xpool", bufs=1) as xpool,
            tc.tile_pool(name="wpool", bufs=2) as wpool,
            tc.tile_pool(name="qkpool", bufs=1) as qkpool,
            tc.tile_pool(name="vpool", bufs=4) as vpool,
            tc.tile_pool(name="epool", bufs=4) as epool,
            tc.tile_pool(name="rpool", bufs=2) as rpool,
            tc.tile_pool(name="opool", bufs=3) as opool,
            tc.tile_pool(name="mm_ps", bufs=2, space="PSUM") as mm_ps,
            tc.tile_pool(name="s_ps", bufs=2, space="PSUM") as s_ps,
            tc.tile_pool(name="o_ps", bufs=2, space="PSUM") as o_ps,
        ):
            # ---------- persistent tiles ----------
            # X tiles split so early projections can start before the full
            # input lands; spread across the three DMA-capable queues.
            XqTs = [xpool.tile([128, NDCQ, 512], BF16, name=f"xqt{i}")
                    for i in range(2)]
            XkvTs = [xpool.tile([128, NDCK, 1024], BF16, name=f"xkvt{i}")
                     for i in range(2)]
            nc.sync.dma_start(out=XkvTs[0][:, 0:3, :],
                              in_=xkv_r[:, 0:3, 0:1024])
            nc.gpsimd.dma_start(out=XkvTs[0][:, 3:6, :],
                                in_=xkv_r[:, 3:6, 0:1024])
            nc.sync.dma_start(out=XkvTs[1][:], in_=xkv_r[:, :, 1024:2048])

            def xq_slice(dc, lo, hi):
                t = lo // 512
                assert hi <= (t + 1) * 512
                return XqTs[t][:, dc, lo - t * 512:hi - t * 512]

            def xkv_slice(dc, lo, hi):
                t = lo // 1024
                assert hi <= (t + 1) * 1024
                return XkvTs[t][:, dc, lo - t * 1024:hi - t * 1024]

            bo_bcast = persist.tile([128, OD], F32)
            bo_ap = bass.AP(tensor=bo[:].tensor, offset=bo[:].offset,
                            ap=[[0, 128]] + list(bo[:].ap))
            nc.gpsimd.dma_start(out=bo_bcast[:], in_=bo_ap)

            wo_t = persist.tile([128, NDCQ, OD], BF16)

            wk0_t = persist.tile([128, NDCK, 128], BF16, name="wk0")
            wv0_t = persist.tile([128, NDCK, 256], BF16, name="wv0")

            qt = [persist.tile([128, LQ], BF16, name=f"qt{p}")
                  for p in range(NPAIR)]
            kt = [persist.tile([128, LKV], BF16, name=f"kt{p}")
                  for p in range(NPAIR)]
            OTn = [persist.tile([128, LQ], BF16, name=f"otn{p}")
                   for p in range(NPAIR)]
            V2 = {}

            # ---------- phase B generator: projections for pair p ----------
            def proj_pair(p):
                # pair 0 runs K/V first: their inputs+weights land well
                # before XqT finishes streaming
                if p == 0:
                    # first pair's K/V weights jump the scalar queue, then
                    # the inputs; K/V projections start ~7us in while XqT
                    # still streams
                    nc.scalar.dma_start(out=wk0_t[:], in_=wk_r[:, :, 0:128])
                    nc.scalar.dma_start(out=wv0_t[:], in_=wv_r[:, :, 0:256])
                    nc.scalar.dma_start(out=XqTs[0][:], in_=xq_r[:, :, 0:512])
                    nc.scalar.dma_start(out=XqTs[1][:],
                                        in_=xq_r[:, :, 512:1024])
                    yield from _proj_k(p)
                    yield from _proj_v(p)
                    yield from _proj_q(p)
                else:
                    yield from _proj_q(p)
                    yield from _proj_k(p)
                    yield from _proj_v(p)

            def _proj_q(p):
                # Q^T pair cols [128, LQ]
                wq_t = wpool.tile([128, NDCQ, 128], BF16, tag="wq")
                nc.scalar.dma_start(out=wq_t[:],
                                    in_=wq_r[:, :, p * 128:(p + 1) * 128])
                if p == 2:
                    # out-proj weights are only needed in phase D
                    nc.gpsimd.dma_start(out=wo_t[:], in_=wo_r)
                for qb in range(NQB):
                    pq = mm_ps.tile([128, 512], F32, tag="mm")
                    for dc in range(NDCQ):
                        nc.tensor.matmul(
                            pq[:], wq_t[:, dc, :],
                            xq_slice(dc, qb * 512, (qb + 1) * 512),
                            start=(dc == 0), stop=(dc == NDCQ - 1))
                        if dc % 4 == 1:
                            yield
                    nc.scalar.copy(qt[p][:, qb * 512:(qb + 1) * 512], pq[:])
                    yield

            def _proj_k(p):
                # K^T pair cols [128, LKV]
                if p == 0:
                    wk_t = wk0_t
                else:
                    wk_t = wpool.tile([128, NDCK, 128], BF16, tag="wk")
                    nc.scalar.dma_start(
                        out=wk_t[:], in_=wk_r[:, :, p * 128:(p + 1) * 128])
                for kb in range(LKV // 512):
                    pk = mm_ps.tile([128, 512], F32, tag="mm")
                    for dc in range(NDCK):
                        nc.tensor.matmul(
                            pk[:], wk_t[:, dc, :],
                            xkv_slice(dc, kb * 512, (kb + 1) * 512),
                            start=(dc == 0), stop=(dc == NDCK - 1))
                        if dc % 3 == 1:
                            yield
                    nc.scalar.copy(kt[p][:, kb * 512:(kb + 1) * 512], pk[:])
                    yield

            def _proj_v(p):
                # V natural [kv, 4 heads x 64] bf16 for the pair-GROUP
                # (built once per even pair, covers pairs p and p+1);
                # ones cols 64:96 per head for the softmax denominator.
                if p % 2 == 1:
                    return
                if p == 0:
                    wv_t = wv0_t
                else:
                    wv_t = wpool.tile([128, NDCK, 256], BF16, tag="wv")
                    nc.scalar.dma_start(
                        out=wv_t[:], in_=wv_r[:, :, p * 128:(p + 2) * 128])
                vts = []
                for x in range(2):
                    v2 = vpool.tile([128, NKC, 2, 128], BF16, tag="v2",
                                    name=f"v2_{p + x}")
                    V2[p + x] = v2
                    vts.append(v2)
                    nc.gpsimd.memset(v2[:, :, :, 64:128], 1.0)
                for kc2 in range(NKC // 2):
                    pv = mm_ps.tile([128, 512], F32, tag="mm")
                    for j in range(2):
                        kc = 2 * kc2 + j
                        for dc in range(NDCK):
                            nc.tensor.matmul(
                                pv[:, j * 256:(j + 1) * 256],
                                xkv_slice(dc, kc * 128, (kc + 1) * 128),
                                wv_t[:, dc, :],
                                start=(dc == 0), stop=(dc == NDCK - 1))
                        yield
                    pv5 = pv[:].rearrange("p (j pr h d) -> p j pr h d",
                                          j=2, pr=2, h=2)
                    for x in range(2):
                        nc.vector.tensor_copy(
                            vts[x][:, kc2 * 2:kc2 * 2 + 2, :, 0:64],
                            pv5[:, :, x, :, :])
                    yield

            # ---------- phase C generator: attention for pair p ----------
            def attn_pair(p, feeder):
                v2 = V2[p]
                ei = 0
                for qb in range(NQB):
                    po = [o_ps.tile([128, 512], F32, tag="o", name=f"po{h}")
                          for h in range(2)]

                    def do_pv(b, e2b):
                        # PV for iteration b (lagged one iter behind S so
                        # the PE never waits on the exp engines)
                        for j in range(2):
                            kc = 2 * b + j
                            st = (b == 0 and j == 0)
                            sp = (b == NB - 1 and j == 1)
                            for h in range(2):
                                nc.tensor.matmul(
                                    po[h][:], v2[:, kc, h, :],
                                    e2b[h][:, j * 512:(j + 1) * 512],
                                    start=st, stop=sp)

                    prev_e2 = None
                    for b in range(NB):
                        s2 = [s_ps.tile([128, 1024], F32, tag="s",
                                        name=f"s{h}") for h in range(2)]
                        e2 = [epool.tile([128, 1024], BF16, tag="e",
                                         name=f"e{h}") for h in range(2)]
                        for j in range(2):
                            kc = 2 * b + j
                            for h in range(2):
                                r0 = h * 64
                                nc.tensor.matmul(
                                    s2[h][:, j * 512:(j + 1) * 512],
                                    kt[p][r0:r0 + 64,
                                          kc * 128:(kc + 1) * 128],
                                    qt[p][r0:r0 + 64,
                                          qb * 512:(qb + 1) * 512],
                                    start=True, stop=True)
                        next(feeder, None)
                        for h in range(2):
                            if ((ei * 2 + h) * 7) % 16 < 7:
                                nc.scalar.activation(e2[h][:], s2[h][:],
                                                     AF.Exp, scale=SCALE)
                            else:
                                nc.vector.tensor_scalar(
                                    e2[h][:].bitcast(U16), s2[h][:],
                                    A16 * SCALE, C16, OP.mult, OP.add)
                        ei += 1
                        if prev_e2 is not None:
                            do_pv(b - 1, prev_e2)
                        prev_e2 = e2
                        next(feeder, None)
                        yield
                    do_pv(NB - 1, prev_e2)
                    # evacuate po to SBUF fast (2 ACT copies per head,
                    # PSUM source so partition offsets may differ) so the
                    # PSUM bank frees quickly; the normalize chain
                    # (bitcast-reciprocal + Newton + mul) then runs on DVE
                    # off the PE critical path on partition-aligned tiles.
                    for h in range(2):
                        ocO = rpool.tile([64, 512], F32, tag="ocO",
                                         name=f"ocO{h}")
                        nc.scalar.copy(ocO[:], po[h][0:64, :])
                        ri = rpool.tile([64, 512], F32, tag="ri",
                                        name=f"ri{h}")
                        # rinv = exp(-ln(r)) on ACT (exp+ln share one table
                        # set, see get_nc); reads r straight from PSUM.
                        nc.scalar.activation(ri[:], po[h][64:128, :], AF.Ln)
                        nc.scalar.activation(ri[:], ri[:], AF.Exp,
                                             scale=-1.0)
                        nc.vector.tensor_tensor(
                            OTn[p][h * 64:(h + 1) * 64,
                                   qb * 512:(qb + 1) * 512],
                            ocO[:], ri[:], OP.mult)
                        next(feeder, None)
                    yield

            def run_all(gen):
                for _ in gen:
                    pass

            # ---------- phase D generator: out projection ----------
            def d_phase(lcs):
                for ob in range(OD // 512):
                    for lc in lcs:
                        pf = mm_ps.tile([128, 512], F32, tag="mm")
                        for fc in range(NDCQ):
                            nc.tensor.matmul(
                                pf[:], OTn[fc][:, lc * 128:(lc + 1) * 128],
                                wo_t[:, fc, ob * 512:(ob + 1) * 512],
                                start=(fc == 0), stop=(fc == NDCQ - 1))
                        o_sb = opool.tile([128, 512], F32, tag="osb")
                        nc.vector.tensor_tensor(
                            o_sb[:], pf[:],
                            bo_bcast[:, ob * 512:(ob + 1) * 512], OP.add)
                        q = [nc.sync, nc.scalar, nc.gpsimd][lc % 3]
                        q.dma_start(
                            out=out[lc * 128:(lc + 1) * 128,
                                    ob * 512:(ob + 1) * 512],
                            in_=o_sb[:])
                        yield

            # ---------- drive pipeline ----------
            run_all(proj_pair(0))
            for p in range(NPAIR):
                if p + 1 < NPAIR:
                    feeder = proj_pair(p + 1)
                else:
                    # last pair: q rows 0:512 of the out-projection only
                    # need qb=0 results; 18 null steps hold them out of the
                    # PE queue until the qb=1 loop (else the queued D MMs
                    # would stall the PE behind unmet OTn[7] deps)
                    import itertools
                    feeder = itertools.chain(
                        iter([None] * 18), d_phase([0, 1, 2, 3]))
                for _ in attn_pair(p, feeder):
                    pass
                run_all(feeder)
            run_all(d_phase([4, 5, 6, 7]))

    return nc


_CACHED = {}


def _force_combined_exp_ln_table(arch):
    # The act-table placement pass picks, per activation, some set
    # containing its function; with both Exp and Ln in use it ping-pongs
    # between 'exp_and_others' and 'natural_log' (a ~2.7us reload each
    # time). Trim the cached table map (indices preserved - they are the
    # act_func_set_id) so Exp and Ln only resolve to the combined
    # 'natural_log_exp_and_others' set.
    import concourse.hw_specs as hw_specs
    tabs = hw_specs.get_activation_tables(arch)
    exp_t = mybir.ActivationFunctionType.Exp
    ln_t = mybir.ActivationFunctionType.Ln
    for name, fns in tabs.items():
        if name != "natural_log_exp_and_others":
            fns.discard(exp_t)
            fns.discard(ln_t)


def get_nc():
    if "nc" not in _CACHED:
        nc = bacc.Bacc("TRN2", target_bir_lowering=False)
        build(nc)
        _force_combined_exp_ln_table(nc.m.arch)
        nc.finalize()
        _CACHED["nc"] = nc
    return _CACHED["nc"]


def make_in_maps(inputs):
    import ml_dtypes
    bf = ml_dtypes.bfloat16
    query = np.asarray(inputs["query"], dtype=np.float32)
    kv = np.asarray(inputs["kv"], dtype=np.float32)
    Wq = np.asarray(inputs["Wq"], dtype=np.float32).astype(bf)
    Wk = np.asarray(inputs["Wk"], dtype=np.float32).astype(bf)
    Wv = np.asarray(inputs["Wv"], dtype=np.float32).astype(bf)
    Wo = np.asarray(inputs["Wo"], dtype=np.float32).astype(bf)
    bo = np.asarray(inputs["bo"], dtype=np.float32)
    in_maps = []
    for c in range(8):
        b, hh = c // 2, c % 2
        xqT = np.ascontiguousarray(
            query[b, hh * LQ:(hh + 1) * LQ, :].T).astype(bf)
        xkvT = np.ascontiguousarray(kv[b].T).astype(bf)
        in_maps.append({
            "xqT": xqT, "xkvT": xkvT,
            "wq": Wq, "wk": Wk, "wv": Wv, "wo": Wo, "bo": bo,
        })
    return in_maps


def assemble_output(results):
    out_full = np.empty((4, 2048, OD), dtype=np.float32)
    for c in range(8):
        b, hh = c // 2, c % 2
        out_full[b, hh * LQ:(hh + 1) * LQ, :] = results[c]["out"]
    return out_full


def kernel(query, kv, Wq, Wk, Wv, Wo, bo, **run_kwargs):
    N, Lq_full, _ = np.asarray(query).shape
    assert (N, Lq_full) == (4, 2048)
    nc = get_nc()
    in_maps = make_in_maps(dict(query=query, kv=kv, Wq=Wq, Wk=Wk, Wv=Wv,
                                Wo=Wo, bo=bo))
    res = run_bass_kernel_spmd(nc, in_maps, core_ids=list(range(8)),
                               **run_kwargs)
    out_full = assemble_output(res.results)
    if run_kwargs:
        kernel.last_result = res
    return out_full
